# revision 17
# baseline (speedup 1.0000x reference)
"""DNA Transport Hamiltonian GNN kernel for Trainium2 (8 NeuronCores).

Builds [8, 2048, 2048] banded Hamiltonians. Sharding: one graph per core;
MLP weights replicated.

v2: on this deployment each DMA *instruction* costs ~15-50us regardless of
size (measured), so the kernel minimizes DMA count: ONE packed input load
(all ten inputs in a single [128, 10788] tensor), ONE whole-output zero-fill
DMA (stride-0 source broadcast over 16 row blocks), and THREE merged band-
window DMAs (blocks 0 / 1-14 / 15). The compute pipeline (two small MLPs on
PE/ACT, band assembly via PE transposes + DVE masked adds) is unchanged from
v1; compute instructions are effectively free here.

Hardcoded problem structure (from the generating module):
  B=8 graphs, 2048 DNA nodes/graph (+2 contact nodes at graph start),
  HID=128, edges per graph: (i, i+d) for d=1..4 -> 8182, d-major order.
"""

import numpy as np

B = 8
ND = 2048            # DNA nodes per graph == H_size
NPG = ND + 2         # nodes per graph incl. 2 contacts
HID = 128
EP = 8182            # edges per graph
EPAD = 8192
NT = ND // 128       # 16 row blocks
OFF = {1: 0, 2: 2047, 3: 4093, 4: 6138}   # start of band d in per-graph edge order
WIN = 136            # band window width: 128 + 2*4

# packed input column layout ([128, C_TOT] f32)
C_EFT = 0                     # 8192 edge features (transposed, zero-padded)
C_XT = C_EFT + EPAD           # 2048 node features (transposed)
C_WO1 = C_XT + ND             # 128
C_WC1 = C_WO1 + HID           # 128
C_BO1 = C_WC1 + HID           # 1
C_BC1 = C_BO1 + 1             # 1
C_W2 = C_BC1 + 1              # 2 (col0=wc2, col1=wo2)
C_MASK = C_W2 + 2             # 144 shifted-identity masks
C_BIASC = C_MASK + 144        # 144 per-block output biases
C_TOT = C_BIASC + 144         # 10788

_PROG = None


def _build_program():
    import concourse.bass as bass
    import concourse.tile as tile
    from concourse.tile import add_dep_helper
    from concourse import mybir
    from contextlib import ExitStack

    f32 = mybir.dt.float32
    Alu = mybir.AluOpType
    Act = mybir.ActivationFunctionType

    nc = bass.Bass(num_swdge_queues=4)

    pk = nc.declare_dram_parameter("pk", [HID, C_TOT], f32, isOutput=False)
    h = nc.declare_dram_parameter("h", [ND, ND], f32, isOutput=True)

    with tile.TileContext(nc) as tc, ExitStack() as ctx:
        cons = ctx.enter_context(tc.tile_pool(name="cons", bufs=1))
        psL1 = ctx.enter_context(tc.tile_pool(name="psL1", bufs=2, space="PSUM"))
        psPers = ctx.enter_context(tc.tile_pool(name="psPers", bufs=1, space="PSUM"))
        # bufs = NT so slots are never reused: avoids WAR release semaphores
        cpool = ctx.enter_context(tc.tile_pool(name="cpool", bufs=NT))

        PK = cons.tile([HID, C_TOT], f32)
        Z2 = cons.tile([32, 16384], f32)    # wide zero source: 64KB descriptors
        H1ET = cons.tile([HID, 4 + EPAD], f32)  # 4 leading zero cols
        H1XT = cons.tile([HID, ND], f32)
        SCRD = cons.tile([1, 2], f32)
        SCRP = cons.tile([1, 2], f32)   # Pool-only scratch

        EFT = PK[:, C_EFT:C_EFT + EPAD]
        XT = PK[:, C_XT:C_XT + ND]
        WO1 = PK[:, C_WO1:C_WO1 + HID]
        WC1 = PK[:, C_WC1:C_WC1 + HID]
        BO1 = PK[:, C_BO1:C_BO1 + 1]
        BC1 = PK[:, C_BC1:C_BC1 + 1]
        W2 = PK[:, C_W2:C_W2 + 2]
        MASK = PK[:, C_MASK:C_MASK + 144]
        BIASC = PK[:, C_BIASC:C_BIASC + 144]

        # Pool memsets a wide zero tile (32 partitions x 64KB rows); ONE
        # DMA then zero-fills ALL of h with 64KB descriptors (8 repeats of
        # the tile). Band windows are overwritten afterwards: a single-wait
        # DVE copy at block 7 (zb below) passes the zero-queue semaphore, so
        # every window DMA (which waits on a later DVE count) is ordered
        # after the background fill.
        zmem = nc.gpsimd.memset(Z2[:], 0.0)
        zero_out = bass.AP(tensor=h, offset=0,
                           ap=[[16384, 32], [32 * 16384, 8], [1, 16384]])
        zero_in = Z2[:].unsqueeze(1).broadcast_to([32, 8, 16384])
        zero_dma = nc.sync.dma_start(zero_out, zero_in)
        zero_dmas = [zero_dma]
        # single packed input load
        pk_dma = nc.sync.dma_start(PK[:], pk[:])

        # first ACT / DVE ops observe the PK-load queue semaphore once;
        # ACT also zeroes H1ET's 4-col pad (read by block 0's lower diags)
        nc.scalar.activation(H1ET[:, 0:4], PK[:, 0:4], Act.Copy,
                             bias=0.0, scale=0.0)
        nc.vector.tensor_copy(SCRD[0:1, 0:1], MASK[0:1, 0:1])
        zpool = nc.gpsimd.memset(SCRP[0:1, 0:1], 0.0)
        add_dep_helper(zpool.ins, zero_dma.ins,
                       reason="Pool observes zero fill before window DMAs")

        lastd = {}

        def l1l2_edges(j):
            ps = psL1.tile([128, 512], f32)
            nc.tensor.matmul(ps[:], WC1, EFT[:, 512 * j:512 * (j + 1)],
                             start=True, stop=True)
            nc.scalar.activation(H1ET[:, 4 + 512 * j:4 + 512 * (j + 1)], ps[:],
                                 Act.Relu, bias=BC1)

        def l1l2_nodes(g):
            ps = psL1.tile([128, 512], f32)
            nc.tensor.matmul(ps[:], WO1, XT[:, 512 * g:512 * (g + 1)],
                             start=True, stop=True)
            return nc.scalar.activation(H1XT[:, 512 * g:512 * (g + 1)], ps[:],
                                        Act.Relu, bias=BO1)

        PSA = psPers.tile([128, 76], f32)   # 72 band cols + spare col 72
        PSB = psPers.tile([128, 76], f32)
        # four window groups -> four SWDGE DMAs (7 blocks = 896 descriptors
        # per DMA, under the 1024-desc per-queue SWDGE carveout)
        GROUPS = [(0, 1), (1, 7), (8, 7), (15, 1)]
        wt = {}
        for t0, nb in GROUPS:
            tile_w = cons.tile([128, nb * WIN], f32, tag=f"wg{t0}")
            for i in range(nb):
                wt[t0 + i] = (tile_w, i * WIN, t0, nb)
        window_dmas = []

        def emit_block(t):
            r0 = 128 * t
            ps = (PSA, PSB)[t % 2]
            c0 = 9 * (t // 2)
            # dummy write to the spare column: absorbs the PSUM-bank release
            # (DVE) semaphore so the real matmuls only wait on ACT
            nc.tensor.matmul(ps[0:1, 72:73], W2[0:1, 0:1], W2[0:1, 0:1],
                             start=True, stop=True)
            # band column c[p, 4+g] = w2 . H1[:, col(p, g)]: one matmul per
            # diagonal with a contiguous 128-col H1 slice as lhsT
            nc.tensor.matmul(ps[:, c0 + 4:c0 + 5], H1XT[:, r0:r0 + 128],
                             W2[:, 1:2], start=True, stop=True)
            for d in range(1, 5):
                s = 4 + OFF[d] + r0
                nc.tensor.matmul(ps[:, c0 + 4 + d:c0 + 5 + d],
                                 H1ET[:, s:s + 128], W2[:, 0:1],
                                 start=True, stop=True)
                lastd['pe'] = nc.tensor.matmul(
                    ps[:, c0 + 4 - d:c0 + 5 - d],
                    H1ET[:, s - d:s - d + 128], W2[:, 0:1],
                    start=True, stop=True)
            c = cpool.tile([128, 9], f32)
            nc.vector.tensor_tensor(c[:], ps[:, c0:c0 + 9],
                                    BIASC[:, 9 * t:9 * t + 9], op=Alu.add)
            tile_w, j0, t0, nb = wt[t]
            wsl = tile_w[:, j0:j0 + WIN]
            nc.vector.tensor_scalar_mul(wsl, MASK[:, 8:8 + WIN], c[:, 0:1])
            for g in range(1, 9):
                lb = nc.vector.scalar_tensor_tensor(
                    wsl, MASK[:, 8 - g:8 - g + WIN], c[:, g:g + 1], wsl,
                    op0=Alu.mult, op1=Alu.add)
            lastd['dve'] = lb
            if t == t0 + nb - 1:
                # group complete: one SWDGE window DMA
                if t0 == 0:
                    wd = nc.gpsimd.dma_start(h[0:128, 0:132], tile_w[:, 4:WIN])
                elif t0 == NT - 1:
                    wd = nc.gpsimd.dma_start(h[r0:r0 + 128, r0 - 4:ND],
                                             tile_w[:, 0:132])
                else:
                    out_ap = bass.AP(
                        tensor=h, offset=128 * t0 * ND + 128 * t0 - 4,
                        ap=[[ND, 128], [128 * ND + 128, nb], [1, WIN]])
                    in_ap = tile_w[:].rearrange("p (b j) -> p b j", j=WIN)
                    wd = nc.gpsimd.dma_start(out_ap, in_ap)
                window_dmas.append(wd)

        # drive: chunk group g covers EFT chunks {g, 4+g, 8+g, 12+g} (one per
        # band region) plus XT chunk g; blocks lag one chunk group since a
        # block's band slice can straddle into the next chunk. Block 0's
        # lower band slices reach back into the previous band region's tail
        # (chunks 7 and 11, group 3), so it goes last.
        for g in range(4):
            for j in (g, 4 + g, 8 + g, 12 + g):
                l1l2_edges(j)
            lastd['act'] = l1l2_nodes(g)
            if g >= 1:
                for t in range(4 * (g - 1), 4 * g):
                    if t != 0:
                        emit_block(t)
        for t in (12, 13, 14, 15, 0):
            emit_block(t)

        # tail: single-wait nops let SP observe every active proc so the
        # kernel-end Drain has its waits elided
        tail = [zmem, pk_dma] + zero_dmas + window_dmas
        tail += [lastd['pe'], lastd['act'], lastd['dve']]
        for dep in tail:
            n = nc.sync.nop(nofuse=True)
            add_dep_helper(n.ins, dep.ins, reason="tail drain wait split")

    return nc


def _get_program():
    global _PROG
    if _PROG is None:
        _PROG = _build_program()
    return _PROG


def _host_prep(inputs):
    nf = np.asarray(inputs["node_features"], dtype=np.float32)
    ef = np.asarray(inputs["edge_features"], dtype=np.float32)
    assert nf.shape == (B * NPG, HID), nf.shape
    assert ef.shape == (B * EP, HID), ef.shape

    wo1 = np.asarray(inputs["Wo1"], np.float32)
    wc1 = np.asarray(inputs["Wc1"], np.float32)
    bo1 = np.asarray(inputs["bo1"], np.float32).reshape(HID, 1)
    bc1 = np.asarray(inputs["bc1"], np.float32).reshape(HID, 1)
    wo2 = np.asarray(inputs["Wo2"], np.float32).reshape(HID)
    wc2 = np.asarray(inputs["Wc2"], np.float32).reshape(HID)
    bo2 = float(np.asarray(inputs["bo2"]).reshape(()))
    bc2 = float(np.asarray(inputs["bc2"]).reshape(()))
    w2 = np.stack([wc2, wo2], axis=1)  # [128, 2]

    # mask0[p, j'] = 1 iff j' == p + 8 ; band-g mask is mask0[:, 8-g : 8-g+136]
    p = np.arange(128)[:, None]
    jp = np.arange(144)[None, :]
    mask0 = (jp == p + 8).astype(np.float32)

    # biasc[p, 9t+g]: +bo2+1e-6 on the diagonal band (g=4), +bc2 on couplings
    row9 = np.array([bc2] * 4 + [bo2 + 1e-6] + [bc2] * 4, np.float32)
    biasc = np.broadcast_to(np.tile(row9, NT), (128, 9 * NT))

    base = np.zeros((HID, C_TOT), np.float32)
    base[:, C_WO1:C_WO1 + HID] = wo1
    base[:, C_WC1:C_WC1 + HID] = wc1
    base[:, C_BO1:C_BO1 + 1] = bo1
    base[:, C_BC1:C_BC1 + 1] = bc1
    base[:, C_W2:C_W2 + 2] = w2
    base[:, C_MASK:C_MASK + 144] = mask0
    base[:, C_BIASC:C_BIASC + 144] = biasc

    in_maps = []
    for b in range(B):
        x_b = nf[b * NPG + 2:(b + 1) * NPG]                    # [2048, 128]
        ef_b = ef[b * EP:(b + 1) * EP]                         # [8182, 128]
        pk = base.copy()
        pk[:, C_EFT:C_EFT + EP] = ef_b.T
        pk[:, C_XT:C_XT + ND] = x_b.T
        in_maps.append({"pk": np.ascontiguousarray(pk)})
    return in_maps


def kernel(**inputs):
    import sys
    if "/opt/trn_rl_repo" not in sys.path:
        sys.path.insert(0, "/opt/trn_rl_repo")
    from concourse.bass_utils import run_bass_kernel_spmd

    nc = _get_program()
    in_maps = _host_prep(inputs)
    res = run_bass_kernel_spmd(nc, in_maps, core_ids=list(range(B)))
    out = np.stack([np.asarray(res.results[i]["h"]) for i in range(B)], axis=0)
    return out.astype(np.float32)


# revision 18
# speedup vs baseline: 1.2349x; 1.2349x over previous
"""DNA Transport Hamiltonian GNN kernel for Trainium2 (8 NeuronCores).

Builds [8, 2048, 2048] banded Hamiltonians. Sharding: one graph per core;
MLP weights replicated.

v2: on this deployment each DMA *instruction* costs ~15-50us regardless of
size (measured), so the kernel minimizes DMA count: ONE packed input load
(all ten inputs in a single [128, 10788] tensor), ONE whole-output zero-fill
DMA (stride-0 source broadcast over 16 row blocks), and THREE merged band-
window DMAs (blocks 0 / 1-14 / 15). The compute pipeline (two small MLPs on
PE/ACT, band assembly via PE transposes + DVE masked adds) is unchanged from
v1; compute instructions are effectively free here.

Hardcoded problem structure (from the generating module):
  B=8 graphs, 2048 DNA nodes/graph (+2 contact nodes at graph start),
  HID=128, edges per graph: (i, i+d) for d=1..4 -> 8182, d-major order.
"""

import numpy as np

B = 8
ND = 2048            # DNA nodes per graph == H_size
NPG = ND + 2         # nodes per graph incl. 2 contacts
HID = 128
EP = 8182            # edges per graph
EPAD = 8192
NT = ND // 128       # 16 row blocks
OFF = {1: 0, 2: 2047, 3: 4093, 4: 6138}   # start of band d in per-graph edge order
WIN = 136            # band window width: 128 + 2*4

# packed input column layout ([128, C_TOT] f32)
C_EFT = 0                     # 8192 edge features (transposed, zero-padded)
C_XT = C_EFT + EPAD           # 2048 node features (transposed)
C_WO1 = C_XT + ND             # 128
C_WC1 = C_WO1 + HID           # 128
C_BO1 = C_WC1 + HID           # 1
C_BC1 = C_BO1 + 1             # 1
C_W2 = C_BC1 + 1              # 2 (col0=wc2, col1=wo2)
C_MASK = C_W2 + 2             # 144 shifted-identity masks
C_BIASC = C_MASK + 144        # 144 per-block output biases
C_TOT = C_BIASC + 144         # 10788

_PROG = None


def _build_program():
    import concourse.bass as bass
    import concourse.tile as tile
    from concourse.tile import add_dep_helper
    from concourse import mybir
    from contextlib import ExitStack

    f32 = mybir.dt.float32
    Alu = mybir.AluOpType
    Act = mybir.ActivationFunctionType

    nc = bass.Bass(num_swdge_queues=4)

    pk = nc.declare_dram_parameter("pk", [HID, C_TOT], f32, isOutput=False)
    h = nc.declare_dram_parameter("h", [ND, ND], f32, isOutput=True)

    with tile.TileContext(nc) as tc, ExitStack() as ctx:
        cons = ctx.enter_context(tc.tile_pool(name="cons", bufs=1))
        psL1 = ctx.enter_context(tc.tile_pool(name="psL1", bufs=2, space="PSUM"))
        psRow = ctx.enter_context(tc.tile_pool(name="psRow", bufs=2, space="PSUM"))
        psPers = ctx.enter_context(tc.tile_pool(name="psPers", bufs=1, space="PSUM"))
        # bufs = NT so slots are never reused: avoids WAR release semaphores
        cpool = ctx.enter_context(tc.tile_pool(name="cpool", bufs=NT))

        PK = cons.tile([HID, C_TOT], f32)
        Z2 = cons.tile([32, 16384], f32)    # wide zero source: 64KB descriptors
        H1ET = cons.tile([HID, EPAD], f32)
        H1XT = cons.tile([HID, ND], f32)
        RE = cons.tile([1, 4 + EPAD], f32)  # coupling row, 4 leading zeros
        RX = cons.tile([1, ND], f32)        # onsite row
        ONE1 = cons.tile([1, 1], f32)
        SCRD = cons.tile([1, 2], f32)
        SCRP = cons.tile([1, 2], f32)   # Pool-only scratch

        EFT = PK[:, C_EFT:C_EFT + EPAD]
        XT = PK[:, C_XT:C_XT + ND]
        WO1 = PK[:, C_WO1:C_WO1 + HID]
        WC1 = PK[:, C_WC1:C_WC1 + HID]
        BO1 = PK[:, C_BO1:C_BO1 + 1]
        BC1 = PK[:, C_BC1:C_BC1 + 1]
        W2 = PK[:, C_W2:C_W2 + 2]
        MASK = PK[:, C_MASK:C_MASK + 144]
        BIASC = PK[:, C_BIASC:C_BIASC + 144]

        # Pool memsets a wide zero tile (32 partitions x 64KB rows); ONE
        # DMA then zero-fills ALL of h with 64KB descriptors (8 repeats of
        # the tile). Band windows are overwritten afterwards: a single-wait
        # DVE copy at block 7 (zb below) passes the zero-queue semaphore, so
        # every window DMA (which waits on a later DVE count) is ordered
        # after the background fill.
        zmem = nc.gpsimd.memset(Z2[:], 0.0)
        zero_out = bass.AP(tensor=h, offset=0,
                           ap=[[16384, 32], [32 * 16384, 8], [1, 16384]])
        zero_in = Z2[:].unsqueeze(1).broadcast_to([32, 8, 16384])
        zero_dma = nc.sync.dma_start(zero_out, zero_in)
        zero_dmas = [zero_dma]
        # single packed input load
        pk_dma = nc.sync.dma_start(PK[:], pk[:])

        # first ACT / DVE ops observe the PK-load queue semaphore once;
        # ACT also makes ONE1 (transpose identity) and RE's 4-col zero pad
        nc.scalar.activation(ONE1[0:1, 0:1], PK[0:1, 0:1], Act.Copy,
                             bias=1.0, scale=0.0)
        nc.scalar.activation(RE[0:1, 0:4], PK[0:1, 0:4], Act.Copy,
                             bias=0.0, scale=0.0)
        nc.vector.tensor_copy(SCRD[0:1, 0:1], MASK[0:1, 0:1])
        zpool = nc.gpsimd.memset(SCRP[0:1, 0:1], 0.0)
        add_dep_helper(zpool.ins, zero_dma.ins,
                       reason="Pool observes zero fill before window DMAs")

        lastd = {}

        def l1l2_edges(j):
            ps = psL1.tile([128, 512], f32)
            nc.tensor.matmul(ps[:], WC1, EFT[:, 512 * j:512 * (j + 1)],
                             start=True, stop=True)
            nc.scalar.activation(H1ET[:, 512 * j:512 * (j + 1)], ps[:],
                                 Act.Relu, bias=BC1)
            ps2 = psRow.tile([1, 512], f32)
            nc.tensor.matmul(ps2[:], W2[:, 0:1],
                             H1ET[:, 512 * j:512 * (j + 1)],
                             start=True, stop=True)
            nc.scalar.copy(RE[0:1, 4 + 512 * j:4 + 512 * (j + 1)], ps2[:])

        def l1l2_nodes(g):
            ps = psL1.tile([128, 512], f32)
            nc.tensor.matmul(ps[:], WO1, XT[:, 512 * g:512 * (g + 1)],
                             start=True, stop=True)
            nc.scalar.activation(H1XT[:, 512 * g:512 * (g + 1)], ps[:],
                                 Act.Relu, bias=BO1)
            ps2 = psRow.tile([1, 512], f32)
            nc.tensor.matmul(ps2[:], W2[:, 1:2],
                             H1XT[:, 512 * g:512 * (g + 1)],
                             start=True, stop=True)
            return nc.scalar.copy(RX[0:1, 512 * g:512 * (g + 1)], ps2[:])

        PSA = psPers.tile([128, 76], f32)   # 72 band cols + spare col 72
        PSB = psPers.tile([128, 76], f32)
        # four window groups -> four SWDGE DMAs (7 blocks = 896 descriptors
        # per DMA, under the 1024-desc per-queue SWDGE carveout)
        GROUPS = [(0, 1), (1, 7), (8, 7), (15, 1)]
        wt = {}
        for t0, nb in GROUPS:
            tile_w = cons.tile([128, nb * WIN], f32, tag=f"wg{t0}")
            for i in range(nb):
                wt[t0 + i] = (tile_w, i * WIN, t0, nb)
        window_dmas = []

        def emit_block(t):
            r0 = 128 * t
            ps = (PSA, PSB)[t % 2]
            c0 = 9 * (t // 2)
            # dummy write to the spare column: absorbs the PSUM-bank release
            # (DVE) semaphore so the real transposes only wait on ACT
            nc.tensor.transpose(ps[0:1, 72:73], ONE1[0:1, 0:1], ONE1[:])
            nc.tensor.transpose(ps[:, c0 + 4:c0 + 5], RX[0:1, r0:r0 + 128], ONE1[:])
            for d in range(1, 5):
                s = 4 + OFF[d] + r0
                nc.tensor.transpose(ps[:, c0 + 4 + d:c0 + 5 + d],
                                    RE[0:1, s:s + 128], ONE1[:])
                lastd['pe'] = nc.tensor.transpose(
                    ps[:, c0 + 4 - d:c0 + 5 - d],
                    RE[0:1, s - d:s - d + 128], ONE1[:])
            c = cpool.tile([128, 9], f32)
            nc.vector.tensor_tensor(c[:], ps[:, c0:c0 + 9],
                                    BIASC[:, 9 * t:9 * t + 9], op=Alu.add)
            tile_w, j0, t0, nb = wt[t]
            wsl = tile_w[:, j0:j0 + WIN]
            nc.vector.tensor_scalar_mul(wsl, MASK[:, 8:8 + WIN], c[:, 0:1])
            for g in range(1, 9):
                lb = nc.vector.scalar_tensor_tensor(
                    wsl, MASK[:, 8 - g:8 - g + WIN], c[:, g:g + 1], wsl,
                    op0=Alu.mult, op1=Alu.add)
            lastd['dve'] = lb
            if t == t0 + nb - 1:
                # group complete: one SWDGE window DMA
                if t0 == 0:
                    wd = nc.gpsimd.dma_start(h[0:128, 0:132], tile_w[:, 4:WIN])
                elif t0 == NT - 1:
                    wd = nc.gpsimd.dma_start(h[r0:r0 + 128, r0 - 4:ND],
                                             tile_w[:, 0:132])
                else:
                    out_ap = bass.AP(
                        tensor=h, offset=128 * t0 * ND + 128 * t0 - 4,
                        ap=[[ND, 128], [128 * ND + 128, nb], [1, WIN]])
                    in_ap = tile_w[:].rearrange("p (b j) -> p b j", j=WIN)
                    wd = nc.gpsimd.dma_start(out_ap, in_ap)
                window_dmas.append(wd)

        # drive: chunk group g covers EFT chunks {g, 4+g, 8+g, 12+g} (one per
        # band region) plus XT chunk g; blocks lag one chunk group since a
        # block's band slice can straddle into the next chunk. Block 0's
        # lower band slices reach back into the previous band region's tail
        # (chunks 7 and 11, group 3), so it goes last.
        for g in range(4):
            for j in (g, 4 + g, 8 + g, 12 + g):
                l1l2_edges(j)
            lastd['act'] = l1l2_nodes(g)
            if g >= 1:
                for t in range(4 * (g - 1), 4 * g):
                    if t != 0:
                        emit_block(t)
        for t in (12, 13, 14, 15, 0):
            emit_block(t)

        # tail: single-wait nops let SP observe every active proc so the
        # kernel-end Drain has its waits elided
        tail = [zmem, pk_dma] + zero_dmas + window_dmas
        tail += [lastd['pe'], lastd['act'], lastd['dve']]
        for dep in tail:
            n = nc.sync.nop(nofuse=True)
            add_dep_helper(n.ins, dep.ins, reason="tail drain wait split")

    return nc


def _get_program():
    global _PROG
    if _PROG is None:
        _PROG = _build_program()
    return _PROG


def _host_prep(inputs):
    nf = np.asarray(inputs["node_features"], dtype=np.float32)
    ef = np.asarray(inputs["edge_features"], dtype=np.float32)
    assert nf.shape == (B * NPG, HID), nf.shape
    assert ef.shape == (B * EP, HID), ef.shape

    wo1 = np.asarray(inputs["Wo1"], np.float32)
    wc1 = np.asarray(inputs["Wc1"], np.float32)
    bo1 = np.asarray(inputs["bo1"], np.float32).reshape(HID, 1)
    bc1 = np.asarray(inputs["bc1"], np.float32).reshape(HID, 1)
    wo2 = np.asarray(inputs["Wo2"], np.float32).reshape(HID)
    wc2 = np.asarray(inputs["Wc2"], np.float32).reshape(HID)
    bo2 = float(np.asarray(inputs["bo2"]).reshape(()))
    bc2 = float(np.asarray(inputs["bc2"]).reshape(()))
    w2 = np.stack([wc2, wo2], axis=1)  # [128, 2]

    # mask0[p, j'] = 1 iff j' == p + 8 ; band-g mask is mask0[:, 8-g : 8-g+136]
    p = np.arange(128)[:, None]
    jp = np.arange(144)[None, :]
    mask0 = (jp == p + 8).astype(np.float32)

    # biasc[p, 9t+g]: +bo2+1e-6 on the diagonal band (g=4), +bc2 on couplings
    row9 = np.array([bc2] * 4 + [bo2 + 1e-6] + [bc2] * 4, np.float32)
    biasc = np.broadcast_to(np.tile(row9, NT), (128, 9 * NT))

    base = np.zeros((HID, C_TOT), np.float32)
    base[:, C_WO1:C_WO1 + HID] = wo1
    base[:, C_WC1:C_WC1 + HID] = wc1
    base[:, C_BO1:C_BO1 + 1] = bo1
    base[:, C_BC1:C_BC1 + 1] = bc1
    base[:, C_W2:C_W2 + 2] = w2
    base[:, C_MASK:C_MASK + 144] = mask0
    base[:, C_BIASC:C_BIASC + 144] = biasc

    in_maps = []
    for b in range(B):
        x_b = nf[b * NPG + 2:(b + 1) * NPG]                    # [2048, 128]
        ef_b = ef[b * EP:(b + 1) * EP]                         # [8182, 128]
        pk = base.copy()
        pk[:, C_EFT:C_EFT + EP] = ef_b.T
        pk[:, C_XT:C_XT + ND] = x_b.T
        in_maps.append({"pk": np.ascontiguousarray(pk)})
    return in_maps


def kernel(**inputs):
    import sys
    if "/opt/trn_rl_repo" not in sys.path:
        sys.path.insert(0, "/opt/trn_rl_repo")
    from concourse.bass_utils import run_bass_kernel_spmd

    nc = _get_program()
    in_maps = _host_prep(inputs)
    res = run_bass_kernel_spmd(nc, in_maps, core_ids=list(range(B)))
    out = np.stack([np.asarray(res.results[i]["h"]) for i in range(B)], axis=0)
    return out.astype(np.float32)
